# revision 3
# baseline (speedup 1.0000x reference)
"""AttentionConv (sparse local attention, 7x7 window, per-channel softmax)
Trainium2 Bass kernel, SPMD across 8 NeuronCores.

Sharding: core i handles batch b = i//2 and channel half cg = i%2
(channels are independent through the whole op: 1x1 convs produce each
output channel from all input channels, and the softmax is per-channel
over the 7x7 window).

The relative-position bias for channels [0,128) is rel_h[u] (window row)
and for channels [128,256) is rel_w[v] (window col). To keep one SPMD
program for all cores, cg=1 cores receive spatially TRANSPOSED x (H<->W)
and their output is transposed back on the host; under that transpose
rel_w becomes a window-row bias, identical in structure to cg=0.

Per-core pipeline (fp16 score path, bf16 value path, f32 accumulate):
  1. DMAs: all 3 weights in one trigger first, then x chunk0 / bias /
     x chunk1 / ident split across the sync+scalar queues.
  2. PE warm-up matmuls chew on a DVE-memset tile (no DMA dependency),
     ramping the PE p-state so the real GEMMs run at full clock.
  3. PE GEMMs q,k,v = W @ x in fp16, interleaved k0,q0,k1,q1,v0,v1 so
     the ACT scatter chain kpE0 -> q0 -> kpE1 -> q1 never starves.
  4. k scattered into a zero-padded plane (ACT); 7 bias-added copies
     kbE[u][r] = kpE[u+r] + b[u] (DVE tensor_scalar 4x mode), u<=3 ones
     split at the GEMM row boundary so they overlap the k-ch1 GEMM.
     The odd-window-col planes are pure element-shifts of the even ones
     (kbO[u][r,c] = kbE[u][r,c+1], vpO[r,c] = vpE[r,c+1]) and are built
     by SBUF->SBUF DMAs -- zero compute-engine cost.
  5. v=0 is emitted with row-chunk-split TTs (16-row pieces) so the
     DVE->ACT->PE pipeline fills ~3us earlier; v=1..5 use the measured
     (4,3) u-split; all elementwise on DVE (TT 2x mode), exp on ACT,
     per-(u,chunk) identity matmuls accumulate num/den in PSUM on PE.
  6. v=6 is chunk-ordered (den/num ch0 fully retired first) so the ch0
     reciprocal + out-mul + DMA overlap the ch1 matmuls.
Steady state is DVE-bound (~7.7us/v); ACT ~6.2us/v; PE ~6.1us/v.
"""

import os

import numpy as np
import ml_dtypes

K = 7
PAD = 3
H = W = 32
HW = H * W
B = 4
C = 256
RS = 40          # padded plane row stride (elements); even => alignment
PR = H + 2 * PAD  # 38 padded rows
N_CORES = 8

_NC_CACHE = {}


def _build_nc():
    import concourse.bass as bass
    import concourse.tile as tile
    from concourse import mybir, bacc

    bf16 = mybir.dt.bfloat16
    f16 = mybir.dt.float16
    f32 = mybir.dt.float32

    nc = bacc.Bacc(None)
    x_ext = nc.dram_tensor("x", [128, 2, HW], f16, kind="ExternalInput")
    w_ext = nc.dram_tensor("w", [128, 3, 2, 128], f16, kind="ExternalInput")
    b_ext = nc.dram_tensor("bias", [128, K], f32, kind="ExternalInput")
    i_ext = nc.dram_tensor("ident", [128, 128], bf16, kind="ExternalInput")
    # fp16 output: halves the output DMA; |out| <= ~7 so fp16 rounding
    # adds ~3e-4 rel err against the 2e-2 budget. Host casts back.
    o_ext = nc.dram_tensor("out", [128, HW], f16, kind="ExternalOutput")

    with tile.TileContext(nc) as tc:
        with (
            tc.tile_pool(name="consts", bufs=1) as consts,
            tc.tile_pool(name="kv", bufs=1) as kv,
            tc.tile_pool(name="fin", bufs=1) as fin,
            tc.tile_pool(name="psa", bufs=1, space="PSUM") as psa,
            tc.tile_pool(name="gt", bufs=1) as gt,
            tc.tile_pool(name="psg", bufs=4, space="PSUM") as psg,
            tc.tile_pool(name="sp", bufs=3) as sp,
            tc.tile_pool(name="ep", bufs=3) as ep,
            tc.tile_pool(name="mp", bufs=3) as mp,
        ):
            xsb = gt.tile([128, 2, HW], f16)
            wsb = gt.tile([128, 3, 2, 128], f16)
            bsb = consts.tile([128, K], f32)
            isb = consts.tile([128, 128], bf16)
            wu = gt.tile([128, 512], f16)  # PE warm-up fodder

            # DMA queue order (each trigger blocks its queue until its
            # source/dest deps are ready, so order = priority):
            #   sync:   w_all, x1, ident, kbO[0,2,4,6], out0
            #   scalar: x0, bias, kbO[1,3,5], vpO, out1
            nc.sync.dma_start(out=wsb[:], in_=w_ext[:])
            nc.scalar.dma_start(out=xsb[:, :, 0:512], in_=x_ext[:, :, 0:512])
            nc.sync.dma_start(out=xsb[:, :, 512:HW], in_=x_ext[:, :, 512:HW])
            nc.scalar.dma_start(out=bsb[:], in_=b_ext[:])
            nc.sync.dma_start(out=isb[:], in_=i_ext[:])

            # q plane (read via a 7-way broadcast AP in the main loop)
            qsb = kv.tile([128, H, W], f16)
            # padded k/v planes; E holds interior at col 3 (for even v
            # window reads); the odd-col (O) variants are one-element
            # shifts built later by SBUF->SBUF DMA.
            kpE = gt.tile([128, PR, RS], f16)
            vpE = kv.tile([128, PR, RS], bf16)
            vpO = kv.tile([128, PR, RS], bf16)
            # kb planes are stored pre-shifted: kb[u][r] = kp[u+r] + b[u]
            # for r in 0..32 (only rows u..u+31 of a padded plane are
            # ever read for window-row u).
            PW32 = H * RS
            kbE = kv.tile([128, K, H, RS], f16)
            kbO = kv.tile([128, K, H, RS], f16)

            nc.vector.memset(wu[:], 0.0)
            nc.gpsimd.memset(kpE[:], 0.0)
            nc.gpsimd.memset(vpE[:], 0.0)

            # per-chunk PSUM accumulators (finer deps => earlier tail)
            nps0 = psa.tile([128, 512], f32)
            nps1 = psa.tile([128, 512], f32)
            dps0 = psa.tile([128, 512], f32)
            dps1 = psa.tile([128, 512], f32)
            nps = [nps0, nps1]
            dps = [dps0, dps1]

            # PE pipeline/HAM warm-up on the memset tile (no DMA dep):
            # ~3us of continuous PE busy ramps the p-state to full clock
            # before the real GEMMs. Content is discarded by the first
            # start=True accumulation.
            nc.tensor.matmul(nps0[:, 0:128], wu[:, 0:128], wu[:, 0:128],
                             start=True, stop=True, skip_group_check=True)
            for _ in range(7):
                nc.tensor.matmul(nps0[:], wu[:, 0:128], wu[:],
                                 start=True, stop=True, skip_group_check=True)

            # ---- GEMMs: wi 0=q, 1=k, 2=v; N chunks of 512 px (16 rows)
            def gemm(wi, ch):
                ps = psg.tile([128, 16, 32], f32, tag="ps", name="ps")
                for ci in range(2):
                    nc.tensor.matmul(
                        ps[:],
                        wsb[:, wi, ci, :],
                        xsb[:, ci, ch * 512:(ch + 1) * 512],
                        start=(ci == 0),
                        stop=(ci == 1),
                    )
                return ps

            # Interleave k/q GEMMs + scatters so the ACT chain
            # kpE0 -> q0 -> kpE1 -> q1 never waits on the PE.
            ps_k0 = gemm(1, 0)
            nc.scalar.copy(kpE[:, PAD:PAD + 16, 3:35], ps_k0[:])
            ps_q0 = gemm(0, 0)
            nc.scalar.copy(qsb[:, 0:16, :], ps_q0[:])
            ps_k1 = gemm(1, 1)
            nc.scalar.copy(kpE[:, PAD + 16:PAD + 32, 3:35], ps_k1[:])
            ps_q1 = gemm(0, 1)
            nc.scalar.copy(qsb[:, 16:32, :], ps_q1[:])
            ps_v = [gemm(2, ch) for ch in range(2)]

            # biased k copies kbE[u] = kpE + b[u] (DVE tensor_scalar 4x).
            # u<=3 split at the GEMM row boundary b0 so the ch0-row
            # copies only wait on the kpE ch0 scatter.
            for u in range(4):
                b0 = PAD + 16 - u
                nc.vector.tensor_scalar_add(
                    kbE[:, u, 0:b0], kpE[:, u:u + b0], bsb[:, u:u + 1])

            def window_ap(t, base_off, u0, nu, u_step, r0, nr):
                full = t[:]
                return bass.AP(
                    tensor=full.tensor,
                    offset=full.offset + base_off + u0 * u_step + r0 * RS,
                    ap=[full.ap[0], [u_step, nu], [RS, nr], [1, W]],
                )

            def q_bcast(nu, r0, nr):
                full = qsb[:]
                return bass.AP(
                    tensor=full.tensor,
                    offset=full.offset + r0 * W,
                    ap=[full.ap[0], [0, nu], [W, nr], [1, W]],
                )

            HALVES = ((0, 4), (4, 3))

            def mk_tiles(tag):
                s4 = sp.tile([128, 4, H, W], f16, tag="s0")
                s3 = sp.tile([128, 3, H, W], f16, tag="s4")
                e4 = ep.tile([128, 4, H, W], bf16, tag="e0")
                e3 = ep.tile([128, 3, H, W], bf16, tag="e4")
                m4 = mp.tile([128, 4, H, W], bf16, tag="m0")
                m3 = mp.tile([128, 3, H, W], bf16, tag="m4")
                return s4, s3, e4, e3, m4, m3

            def mm(acc, t, du, ch, start, stop):
                nc.tensor.matmul(
                    acc[:], isb[:], t[:, du, 16 * ch:16 * ch + 16, :],
                    start=start, stop=stop, skip_group_check=True)

            def mms_for(v, t_by_half, kind, ch, urange):
                acc = (dps if kind == "den" else nps)[ch]
                for u in urange:
                    h = 0 if u < 4 else 1
                    t = t_by_half[h]
                    mm(acc, t, u - 4 * h, ch, v == 0 and u == 0, v == K - 1 and u == K - 1)

            # ================= v = 0 (pipeline fill, row-chunk split) ====
            s4, s3, e4, e3, m4, m3 = mk_tiles("v0")
            Exp = mybir.ActivationFunctionType.Exp

            # s4a: output rows 0..15, u 0..3 -- needs only the ch0-row
            # kb copies above plus the q ch0 scatter.
            nc.vector.tensor_mul(
                s4[:, :, 0:16, :], q_bcast(4, 0, 16),
                window_ap(kbE, 0, 0, 4, PW32, 0, 16))
            nc.scalar.activation(e4[:, :, 0:16, :], s4[:, :, 0:16, :], Exp)
            # remaining rows of kbE[0..3]
            for u in range(4):
                b0 = PAD + 16 - u
                nc.vector.tensor_scalar_add(
                    kbE[:, u, b0:H], kpE[:, u + b0:u + H], bsb[:, u:u + 1])
            # v scatters (ACT, after the first exp in queue order)
            nc.scalar.copy(vpE[:, PAD:PAD + 16, 3:35], ps_v[0][:])
            nc.vector.tensor_mul(
                s4[:, :, 16:32, :], q_bcast(4, 16, 16),
                window_ap(kbE, 0, 0, 4, PW32, 16, 16))
            nc.scalar.activation(e4[:, :, 16:32, :], s4[:, :, 16:32, :], Exp)
            nc.scalar.copy(vpE[:, PAD + 16:PAD + 32, 3:35], ps_v[1][:])
            # kbE[4..6] full planes
            for u in range(4, K):
                nc.vector.tensor_scalar_add(
                    kbE[:, u], kpE[:, u:u + H], bsb[:, u:u + 1])
            # odd-col planes: pure one-element shifts, built by DMA.
            # kbO[u][:, c] = kbE[u][:, c+1]; col 39 is never read.
            nc.sync.dma_start(out=kbO[:, 0, :, 0:RS - 1], in_=kbE[:, 0, :, 1:RS])
            nc.scalar.dma_start(out=kbO[:, 1, :, 0:RS - 1], in_=kbE[:, 1, :, 1:RS])
            nc.sync.dma_start(out=kbO[:, 2, :, 0:RS - 1], in_=kbE[:, 2, :, 1:RS])
            nc.scalar.dma_start(out=kbO[:, 3, :, 0:RS - 1], in_=kbE[:, 3, :, 1:RS])
            nc.sync.dma_start(out=kbO[:, 4, :, 0:RS - 1], in_=kbE[:, 4, :, 1:RS])
            nc.scalar.dma_start(out=kbO[:, 5, :, 0:RS - 1], in_=kbE[:, 5, :, 1:RS])
            nc.sync.dma_start(out=kbO[:, 6, :, 0:RS - 1], in_=kbE[:, 6, :, 1:RS])

            # s3: u 4..6 (full width; kbE[4..6] just built on this queue)
            nc.vector.tensor_mul(
                s3[:], q_bcast(3, 0, H), window_ap(kbE, 0, 4, 3, PW32, 0, H))
            nc.scalar.activation(e3[:], s3[:], Exp)
            nc.scalar.dma_start(out=vpO[:, :, 0:RS - 1], in_=vpE[:, :, 1:RS])

            # m pieces + matmuls per chunk
            nc.vector.tensor_mul(
                m4[:, :, 0:16, :], e4[:, :, 0:16, :],
                window_ap(vpE, 0, 0, 4, RS, 0, 16))
            for kind in ("den", "num"):
                mms_for(0, (e4, e3) if kind == "den" else (m4, m3), kind, 0, range(4))
            nc.vector.tensor_mul(
                m4[:, :, 16:32, :], e4[:, :, 16:32, :],
                window_ap(vpE, 0, 0, 4, RS, 16, 16))
            for kind in ("den", "num"):
                mms_for(0, (e4, e3) if kind == "den" else (m4, m3), kind, 1, range(4))
            nc.vector.tensor_mul(m3[:], e3[:], window_ap(vpE, 0, 4, 3, RS, 0, H))
            for ch in range(2):
                for kind in ("den", "num"):
                    mms_for(0, (e4, e3) if kind == "den" else (m4, m3), kind, ch, range(4, K))

            # ================= v = 1..5 (steady state) ===================
            for v in range(1, K - 1):
                par = v & 1
                kb = kbO if par else kbE
                vp = vpO if par else vpE
                off = v - par  # even

                t_by_half = {}
                for u0, nu in HALVES:
                    s = sp.tile([128, nu, H, W], f16, tag=f"s{u0}")
                    nc.vector.tensor_mul(
                        s[:], q_bcast(nu, 0, H),
                        window_ap(kb, off, u0, nu, PW32, 0, H))
                    e = ep.tile([128, nu, H, W], bf16, tag=f"e{u0}")
                    nc.scalar.activation(e[:], s[:], Exp)
                    m = mp.tile([128, nu, H, W], bf16, tag=f"m{u0}")
                    nc.vector.tensor_mul(
                        m[:], e[:], window_ap(vp, off, u0, nu, RS, 0, H))
                    t_by_half[u0] = (e, m)
                for u0, nu in HALVES:
                    e, m = t_by_half[u0]
                    for du in range(nu):
                        u = u0 + du
                        for ch in range(2):
                            for kind in ("den", "num"):
                                t = e if kind == "den" else m
                                mm((dps if kind == "den" else nps)[ch],
                                   t, du, ch, False, False)

            # ================= v = 6 (tail, chunk-ordered drain) =========
            v = K - 1  # v=6 is even: kb=kbE, vp=vpE, off=6
            off = 6
            s4, s3, e4, e3, m4, m3 = mk_tiles("v6")
            rden = fin.tile([128, HW], f32)
            outsb = fin.tile([128, HW], f16)

            nc.vector.tensor_mul(
                s4[:, :, 0:16, :], q_bcast(4, 0, 16),
                window_ap(kbE, off, 0, 4, PW32, 0, 16))
            nc.scalar.activation(e4[:, :, 0:16, :], s4[:, :, 0:16, :], Exp)
            nc.vector.tensor_mul(
                s3[:, :, 0:16, :], q_bcast(3, 0, 16),
                window_ap(kbE, off, 4, 3, PW32, 0, 16))
            nc.scalar.activation(e3[:, :, 0:16, :], s3[:, :, 0:16, :], Exp)
            nc.vector.tensor_mul(
                s4[:, :, 16:32, :], q_bcast(4, 16, 16),
                window_ap(kbE, off, 0, 4, PW32, 16, 16))
            nc.scalar.activation(e4[:, :, 16:32, :], s4[:, :, 16:32, :], Exp)
            nc.vector.tensor_mul(
                m4[:, :, 0:16, :], e4[:, :, 0:16, :],
                window_ap(vpE, off, 0, 4, RS, 0, 16))
            mms_for(v, (e4, e3), "den", 0, range(0, 4))
            nc.vector.tensor_mul(
                m3[:, :, 0:16, :], e3[:, :, 0:16, :],
                window_ap(vpE, off, 4, 3, RS, 0, 16))
            mms_for(v, (e4, e3), "den", 0, range(4, K))
            mms_for(v, (m4, m3), "num", 0, range(0, 4))
            nc.vector.tensor_mul(
                s3[:, :, 16:32, :], q_bcast(3, 16, 16),
                window_ap(kbE, off, 4, 3, PW32, 16, 16))
            nc.scalar.activation(e3[:, :, 16:32, :], s3[:, :, 16:32, :], Exp)
            mms_for(v, (m4, m3), "num", 0, range(4, K))
            nc.vector.reciprocal_approx_fast(out=rden[:, 0:512], in_=dps[0][:])
            nc.vector.tensor_mul(
                m4[:, :, 16:32, :], e4[:, :, 16:32, :],
                window_ap(vpE, off, 0, 4, RS, 16, 16))
            mms_for(v, (e4, e3), "den", 1, range(0, 4))
            nc.vector.tensor_mul(outsb[:, 0:512], nps[0][:], rden[:, 0:512])
            nc.sync.dma_start(out=o_ext[:, 0:512], in_=outsb[:, 0:512])
            nc.vector.tensor_mul(
                m3[:, :, 16:32, :], e3[:, :, 16:32, :],
                window_ap(vpE, off, 4, 3, RS, 16, 16))
            mms_for(v, (e4, e3), "den", 1, range(4, K))
            mms_for(v, (m4, m3), "num", 1, range(0, 4))
            mms_for(v, (m4, m3), "num", 1, range(4, K))
            nc.vector.reciprocal_approx_fast(out=rden[:, 512:HW], in_=dps[1][:])
            nc.vector.tensor_mul(outsb[:, 512:HW], nps[1][:], rden[:, 512:HW])
            nc.scalar.dma_start(out=o_ext[:, 512:HW], in_=outsb[:, 512:HW])

    nc.finalize()
    return nc


def _get_nc():
    if "nc" not in _NC_CACHE:
        _NC_CACHE["nc"] = _build_nc()
    return _NC_CACHE["nc"]


def _prep_in_maps(x, wq, wk, wv, rel_h, rel_w):
    bf = ml_dtypes.bfloat16
    ident = np.eye(128, dtype=bf)
    in_maps = []
    for core in range(N_CORES):
        b, cg = divmod(core, 2)
        xb = np.asarray(x[b], dtype=np.float32)
        if cg == 1:
            xb = xb.transpose(0, 2, 1)
        # [ci_within_chunk, ci_chunk, px] so the SBUF partition dim is
        # outermost in DRAM (batched DMAs copy AP-order to AP-order)
        xb = np.ascontiguousarray(
            xb.reshape(2, 128, HW).transpose(1, 0, 2)).astype(np.float16)
        rows = slice(cg * 128, (cg + 1) * 128)
        wt = np.stack([np.asarray(wq)[rows], np.asarray(wk)[rows],
                       np.asarray(wv)[rows]])          # [3, 128, 256]
        wt = wt.transpose(0, 2, 1).astype(np.float16).reshape(
            3, 2, 128, 128)                            # [wi, ci_chunk, ci, co]
        wt = np.ascontiguousarray(
            wt.transpose(2, 0, 1, 3))                  # [ci, wi, ci_chunk, co]
        bias = np.ascontiguousarray(
            np.asarray(rel_h if cg == 0 else rel_w, dtype=np.float32))
        in_maps.append({"x": xb, "w": wt, "bias": bias, "ident": ident})
    return in_maps


def _assemble(results):
    out = np.empty((B, C, H, W), np.float32)
    for core in range(N_CORES):
        b, cg = divmod(core, 2)
        o = results[core]["out"].reshape(128, H, W)
        if cg == 1:
            o = o.transpose(0, 2, 1)
        out[b, cg * 128:(cg + 1) * 128] = o
    return out


def run(inputs, trace=False):
    """Returns (output, BassKernelResults)."""
    from concourse import bass_utils

    nc = _get_nc()
    in_maps = _prep_in_maps(**inputs)
    last_err = None
    for _attempt in range(3):
        try:
            res = bass_utils.run_bass_kernel_spmd(
                nc, in_maps, core_ids=list(range(N_CORES)), trace=trace)
            return _assemble(res.results), res
        except Exception as err:  # transient NRT device errors
            last_err = err
    raise last_err


def kernel(x, wq, wk, wv, rel_h, rel_w):
    out, _ = run(
        dict(x=x, wq=wq, wk=wk, wv=wv, rel_h=rel_h, rel_w=rel_w),
        trace=bool(os.environ.get("ATTNCONV_TRACE")),
    )
    return out


# revision 7
# speedup vs baseline: 1.1898x; 1.1898x over previous
"""AttentionConv (sparse local attention, 7x7 window, per-channel softmax)
Trainium2 Bass kernel, SPMD across 8 NeuronCores.

Sharding: core i handles batch b = i//2 and channel half cg = i%2
(channels are independent through the whole op: 1x1 convs produce each
output channel from all input channels, and the softmax is per-channel
over the 7x7 window).

The relative-position bias for channels [0,128) is rel_h[u] (window row)
and for channels [128,256) is rel_w[v] (window col). To keep one SPMD
program for all cores, cg=1 cores receive spatially TRANSPOSED x (H<->W)
and their output is transposed back on the host; under that transpose
rel_w becomes a window-row bias, identical in structure to cg=0.

Per-core pipeline (fp16 score path, bf16 value path, f32 accumulate):
  1. DMAs: all 3 weights in one trigger first, then x chunk0 / bias /
     x chunk1 / ident split across the sync+scalar queues.
  2. PE warm-up matmuls chew on a DVE-memset tile (no DMA dependency),
     ramping the PE p-state so the real GEMMs run at full clock.
  3. PE GEMMs q,k,v = W @ x in fp16, interleaved k0,q0,k1,q1,v0,v1 so
     the ACT scatter chain kpE0 -> q0 -> kpE1 -> q1 never starves.
  4. k scattered into a zero-padded plane (ACT); 7 bias-added copies
     kbE[u][r] = kpE[u+r] + b[u] (DVE tensor_scalar 4x mode), u<=3 ones
     split at the GEMM row boundary so they overlap the k-ch1 GEMM.
     The odd-window-col planes are pure element-shifts of the even ones
     (kbO[u][r,c] = kbE[u][r,c+1], vpO[r,c] = vpE[r,c+1]) and are built
     by SBUF->SBUF DMAs -- zero compute-engine cost.
  5. v=0 is emitted with row-chunk-split TTs (16-row pieces) so the
     DVE->ACT->PE pipeline fills ~3us earlier; v=1..5 use the measured
     (4,3) u-split; all elementwise on DVE (TT 2x mode), exp on ACT,
     per-(u,chunk) identity matmuls accumulate num/den in PSUM on PE.
  6. v=6 is chunk-ordered (den/num ch0 fully retired first) so the ch0
     reciprocal + out-mul + DMA overlap the ch1 matmuls.
Steady state is DVE-bound (~7.7us/v); ACT ~6.2us/v; PE ~6.1us/v.
"""

import os

import numpy as np
import ml_dtypes

K = 7
PAD = 3
H = W = 32
HW = H * W
B = 4
C = 256
RS = 40          # padded plane row stride (elements); even => alignment
PR = H + 2 * PAD  # 38 padded rows
N_CORES = 8

_NC_CACHE = {}


def _build_nc():
    import concourse.bass as bass
    import concourse.tile as tile
    from concourse import mybir, bacc

    bf16 = mybir.dt.bfloat16
    f16 = mybir.dt.float16
    f32 = mybir.dt.float32

    nc = bacc.Bacc(None)
    x_ext = nc.dram_tensor("x", [128, 2, HW], f16, kind="ExternalInput")
    w_ext = nc.dram_tensor("w", [128, 3, 2, 128], f16, kind="ExternalInput")
    b_ext = nc.dram_tensor("bias", [128, K], f32, kind="ExternalInput")
    i_ext = nc.dram_tensor("ident", [128, 128], bf16, kind="ExternalInput")
    # fp16 output: halves the output DMA; |out| <= ~7 so fp16 rounding
    # adds ~3e-4 rel err against the 2e-2 budget. Host casts back.
    o_ext = nc.dram_tensor("out", [128, HW], f16, kind="ExternalOutput")

    with tile.TileContext(nc) as tc:
        with (
            tc.tile_pool(name="consts", bufs=1) as consts,
            tc.tile_pool(name="kv", bufs=1) as kv,
            tc.tile_pool(name="fin", bufs=1) as fin,
            tc.tile_pool(name="psa", bufs=1, space="PSUM") as psa,
            tc.tile_pool(name="gt", bufs=1) as gt,
            tc.tile_pool(name="psg", bufs=4, space="PSUM") as psg,
            tc.tile_pool(name="sp", bufs=3) as sp,
            tc.tile_pool(name="ep", bufs=3) as ep,
            tc.tile_pool(name="mp", bufs=3) as mp,
        ):
            xsb = gt.tile([128, 2, HW], f16)
            wsb = gt.tile([128, 3, 2, 128], f16)
            bsb = consts.tile([128, K], f32)
            isb = consts.tile([128, 128], bf16)
            wu = gt.tile([128, 512], f16)  # PE warm-up fodder

            # DMA queue order (each trigger blocks its queue until its
            # source/dest deps are ready, so order = priority). x0 gates
            # the whole k-GEMM -> scatter -> kb-copy -> first-TT chain,
            # so it rides the sync queue whose first trigger issues
            # earliest; the scalar queue's first slot is behind the
            # ACT_TABLE_LOAD dispatch.
            #   sync:   x0, x1, ident, kbO[0,2,4,6], out0
            #   scalar: w_all, bias, kbO[1,3,5], vpO, out1
            nc.sync.dma_start(out=xsb[:, :, 0:512], in_=x_ext[:, :, 0:512])
            nc.scalar.dma_start(out=wsb[:], in_=w_ext[:])
            nc.sync.dma_start(out=xsb[:, :, 512:HW], in_=x_ext[:, :, 512:HW])
            nc.scalar.dma_start(out=bsb[:], in_=b_ext[:])
            nc.sync.dma_start(out=isb[:], in_=i_ext[:])

            # q plane (read via a 7-way broadcast AP in the main loop)
            qsb = kv.tile([128, H, W], f16)
            # padded k/v planes; E holds interior at col 3 (for even v
            # window reads); the odd-col (O) variants are one-element
            # shifts built later by SBUF->SBUF DMA.
            kpE = gt.tile([128, PR, RS], f16)
            vpE = kv.tile([128, PR, RS], bf16)
            vpO = kv.tile([128, PR, RS], bf16)
            # kb planes are stored pre-shifted: kb[u][r] = kp[u+r] + b[u]
            # for r in 0..32 (only rows u..u+31 of a padded plane are
            # ever read for window-row u).
            PW32 = H * RS
            kbE = kv.tile([128, K, H, RS], f16)
            kbO = kv.tile([128, K, H, RS], f16)

            # wu on GpSimd: its queue reaches real work ~1us before DVE
            # (which is stuck behind its instruction fetch), so the PE
            # warm-ups start earlier.
            nc.gpsimd.memset(wu[:], 0.0)
            nc.gpsimd.memset(kpE[:], 0.0)
            nc.gpsimd.memset(vpE[:], 0.0)

            # per-chunk PSUM accumulators (finer deps => earlier tail)
            nps0 = psa.tile([128, 512], f32)
            nps1 = psa.tile([128, 512], f32)
            dps0 = psa.tile([128, 512], f32)
            dps1 = psa.tile([128, 512], f32)
            nps = [nps0, nps1]
            dps = [dps0, dps1]

            # PE pipeline/HAM warm-up on the memset tile (no DMA dep):
            # ~3us of continuous PE busy ramps the p-state to full clock
            # before the real GEMMs. Content is discarded by the first
            # start=True accumulation.
            nc.tensor.matmul(nps0[:, 0:128], wu[:, 0:128], wu[:, 0:128],
                             start=True, stop=True, skip_group_check=True)
            for _ in range(7):
                nc.tensor.matmul(nps0[:], wu[:, 0:128], wu[:],
                                 start=True, stop=True, skip_group_check=True)

            # ---- GEMMs: wi 0=q, 1=k, 2=v; N chunks of 512 px (16 rows)
            def gemm(wi, ch):
                ps = psg.tile([128, 16, 32], f32, tag="ps", name="ps")
                for ci in range(2):
                    nc.tensor.matmul(
                        ps[:],
                        wsb[:, wi, ci, :],
                        xsb[:, ci, ch * 512:(ch + 1) * 512],
                        start=(ci == 0),
                        stop=(ci == 1),
                    )
                return ps

            # Interleave k/q GEMMs + scatters so the ACT chain
            # kpE0 -> q0 -> kpE1 -> q1 never waits on the PE.
            ps_k0 = gemm(1, 0)
            nc.scalar.copy(kpE[:, PAD:PAD + 16, 3:35], ps_k0[:])
            ps_q0 = gemm(0, 0)
            nc.scalar.copy(qsb[:, 0:16, :], ps_q0[:])
            ps_k1 = gemm(1, 1)
            nc.scalar.copy(kpE[:, PAD + 16:PAD + 32, 3:35], ps_k1[:])
            ps_q1 = gemm(0, 1)
            nc.scalar.copy(qsb[:, 16:32, :], ps_q1[:])
            ps_v = [gemm(2, ch) for ch in range(2)]

            # biased k copies kbE[u] = kpE + b[u] (DVE tensor_scalar 4x).
            # u<=3 split at the GEMM row boundary b0 so the ch0-row
            # copies only wait on the kpE ch0 scatter.
            for u in range(4):
                b0 = PAD + 16 - u
                nc.vector.tensor_scalar_add(
                    kbE[:, u, 0:b0], kpE[:, u:u + b0], bsb[:, u:u + 1])

            def window_ap(t, base_off, u0, nu, u_step, r0, nr):
                full = t[:]
                return bass.AP(
                    tensor=full.tensor,
                    offset=full.offset + base_off + u0 * u_step + r0 * RS,
                    ap=[full.ap[0], [u_step, nu], [RS, nr], [1, W]],
                )

            def q_bcast(nu, r0, nr):
                full = qsb[:]
                return bass.AP(
                    tensor=full.tensor,
                    offset=full.offset + r0 * W,
                    ap=[full.ap[0], [0, nu], [W, nr], [1, W]],
                )

            HALVES = ((0, 4), (4, 3))

            def mk_tiles(tag):
                s4 = sp.tile([128, 4, H, W], f16, tag="s0")
                s3 = sp.tile([128, 3, H, W], f16, tag="s4")
                e4 = ep.tile([128, 4, H, W], bf16, tag="e0")
                e3 = ep.tile([128, 3, H, W], bf16, tag="e4")
                m4 = mp.tile([128, 4, H, W], bf16, tag="m0")
                m3 = mp.tile([128, 3, H, W], bf16, tag="m4")
                return s4, s3, e4, e3, m4, m3

            def mm(acc, t, du, ch, start, stop):
                nc.tensor.matmul(
                    acc[:], isb[:], t[:, du, 16 * ch:16 * ch + 16, :],
                    start=start, stop=stop, skip_group_check=True)

            def mms_for(v, t_by_half, kind, ch, urange):
                acc = (dps if kind == "den" else nps)[ch]
                for u in urange:
                    h = 0 if u < 4 else 1
                    t = t_by_half[h]
                    mm(acc, t, u - 4 * h, ch, v == 0 and u == 0, v == K - 1 and u == K - 1)

            # ================= v = 0 (pipeline fill, row-chunk split) ====
            s4, s3, e4, e3, m4, m3 = mk_tiles("v0")
            Exp = mybir.ActivationFunctionType.Exp

            # s4a: output rows 0..15, u 0..3 -- needs only the ch0-row
            # kb copies above plus the q ch0 scatter.
            nc.vector.tensor_mul(
                s4[:, :, 0:16, :], q_bcast(4, 0, 16),
                window_ap(kbE, 0, 0, 4, PW32, 0, 16))
            nc.scalar.activation(e4[:, :, 0:16, :], s4[:, :, 0:16, :], Exp)
            # remaining rows of kbE[0..3]
            for u in range(4):
                b0 = PAD + 16 - u
                nc.vector.tensor_scalar_add(
                    kbE[:, u, b0:H], kpE[:, u + b0:u + H], bsb[:, u:u + 1])
            # v scatters (ACT, after the first exp in queue order)
            nc.scalar.copy(vpE[:, PAD:PAD + 16, 3:35], ps_v[0][:])
            nc.vector.tensor_mul(
                s4[:, :, 16:32, :], q_bcast(4, 16, 16),
                window_ap(kbE, 0, 0, 4, PW32, 16, 16))
            nc.scalar.activation(e4[:, :, 16:32, :], s4[:, :, 16:32, :], Exp)
            nc.scalar.copy(vpE[:, PAD + 16:PAD + 32, 3:35], ps_v[1][:])
            # kbE[4..6] full planes
            for u in range(4, K):
                nc.vector.tensor_scalar_add(
                    kbE[:, u], kpE[:, u:u + H], bsb[:, u:u + 1])
            # odd-col planes: pure one-element shifts, built by DMA.
            # Copied as ONE flat contiguous run per partition (a row-wise
            # AP would mean 32 tiny 78B descriptors -> several us per
            # DMA). In flat index, col 39 of row r picks up row r+1's
            # col 0 -- garbage, but cols >=36 are never read.
            def flat_shift(eng, dst, src, n):
                out_ap = bass.AP(tensor=dst.tensor, offset=dst.offset,
                                 ap=[dst.ap[0], [1, n]])
                in_ap = bass.AP(tensor=src.tensor, offset=src.offset + 1,
                                ap=[src.ap[0], [1, n]])
                eng.dma_start(out=out_ap, in_=in_ap)

            NP1 = H * RS - 1
            flat_shift(nc.sync, kbO[:, 0], kbE[:, 0], NP1)
            flat_shift(nc.scalar, kbO[:, 1], kbE[:, 1], NP1)
            flat_shift(nc.sync, kbO[:, 2], kbE[:, 2], NP1)
            flat_shift(nc.scalar, kbO[:, 3], kbE[:, 3], NP1)
            flat_shift(nc.sync, kbO[:, 4], kbE[:, 4], NP1)
            flat_shift(nc.scalar, kbO[:, 5], kbE[:, 5], NP1)
            flat_shift(nc.sync, kbO[:, 6], kbE[:, 6], NP1)

            # s3: u 4..6 (full width; kbE[4..6] just built on this queue)
            nc.vector.tensor_mul(
                s3[:], q_bcast(3, 0, H), window_ap(kbE, 0, 4, 3, PW32, 0, H))
            nc.scalar.activation(e3[:], s3[:], Exp)
            flat_shift(nc.scalar, vpO[:], vpE[:], PR * RS - 1)

            # m pieces + matmuls per chunk
            nc.vector.tensor_mul(
                m4[:, :, 0:16, :], e4[:, :, 0:16, :],
                window_ap(vpE, 0, 0, 4, RS, 0, 16))
            for kind in ("den", "num"):
                mms_for(0, (e4, e3) if kind == "den" else (m4, m3), kind, 0, range(4))
            nc.vector.tensor_mul(
                m4[:, :, 16:32, :], e4[:, :, 16:32, :],
                window_ap(vpE, 0, 0, 4, RS, 16, 16))
            for kind in ("den", "num"):
                mms_for(0, (e4, e3) if kind == "den" else (m4, m3), kind, 1, range(4))
            nc.vector.tensor_mul(m3[:], e3[:], window_ap(vpE, 0, 4, 3, RS, 0, H))
            for ch in range(2):
                for kind in ("den", "num"):
                    mms_for(0, (e4, e3) if kind == "den" else (m4, m3), kind, ch, range(4, K))

            # ================= v = 1..5 (steady state) ===================
            for v in range(1, K - 1):
                par = v & 1
                kb = kbO if par else kbE
                vp = vpO if par else vpE
                off = v - par  # even

                t_by_half = {}
                for u0, nu in HALVES:
                    s = sp.tile([128, nu, H, W], f16, tag=f"s{u0}")
                    nc.vector.tensor_mul(
                        s[:], q_bcast(nu, 0, H),
                        window_ap(kb, off, u0, nu, PW32, 0, H))
                    e = ep.tile([128, nu, H, W], bf16, tag=f"e{u0}")
                    nc.scalar.activation(e[:], s[:], Exp)
                    m = mp.tile([128, nu, H, W], bf16, tag=f"m{u0}")
                    nc.vector.tensor_mul(
                        m[:], e[:], window_ap(vp, off, u0, nu, RS, 0, H))
                    t_by_half[u0] = (e, m)
                for u0, nu in HALVES:
                    e, m = t_by_half[u0]
                    for du in range(nu):
                        u = u0 + du
                        for ch in range(2):
                            for kind in ("den", "num"):
                                t = e if kind == "den" else m
                                mm((dps if kind == "den" else nps)[ch],
                                   t, du, ch, False, False)

            # ================= v = 6 (tail, chunk-ordered drain) =========
            v = K - 1  # v=6 is even: kb=kbE, vp=vpE, off=6
            off = 6
            s4, s3, e4, e3, m4, m3 = mk_tiles("v6")
            rden = fin.tile([128, HW], f32)
            outsb = fin.tile([128, HW], f16)

            nc.vector.tensor_mul(
                s4[:, :, 0:16, :], q_bcast(4, 0, 16),
                window_ap(kbE, off, 0, 4, PW32, 0, 16))
            nc.scalar.activation(e4[:, :, 0:16, :], s4[:, :, 0:16, :], Exp)
            nc.vector.tensor_mul(
                s3[:, :, 0:16, :], q_bcast(3, 0, 16),
                window_ap(kbE, off, 4, 3, PW32, 0, 16))
            nc.scalar.activation(e3[:, :, 0:16, :], s3[:, :, 0:16, :], Exp)
            nc.vector.tensor_mul(
                s4[:, :, 16:32, :], q_bcast(4, 16, 16),
                window_ap(kbE, off, 0, 4, PW32, 16, 16))
            nc.scalar.activation(e4[:, :, 16:32, :], s4[:, :, 16:32, :], Exp)
            nc.vector.tensor_mul(
                m4[:, :, 0:16, :], e4[:, :, 0:16, :],
                window_ap(vpE, off, 0, 4, RS, 0, 16))
            mms_for(v, (e4, e3), "den", 0, range(0, 4))
            nc.vector.tensor_mul(
                m3[:, :, 0:16, :], e3[:, :, 0:16, :],
                window_ap(vpE, off, 4, 3, RS, 0, 16))
            mms_for(v, (e4, e3), "den", 0, range(4, K))
            mms_for(v, (m4, m3), "num", 0, range(0, 4))
            nc.vector.tensor_mul(
                s3[:, :, 16:32, :], q_bcast(3, 16, 16),
                window_ap(kbE, off, 4, 3, PW32, 16, 16))
            nc.scalar.activation(e3[:, :, 16:32, :], s3[:, :, 16:32, :], Exp)
            mms_for(v, (m4, m3), "num", 0, range(4, K))
            nc.vector.reciprocal_approx_fast(out=rden[:, 0:512], in_=dps[0][:])
            nc.vector.tensor_mul(
                m4[:, :, 16:32, :], e4[:, :, 16:32, :],
                window_ap(vpE, off, 0, 4, RS, 16, 16))
            mms_for(v, (e4, e3), "den", 1, range(0, 4))
            nc.vector.tensor_mul(outsb[:, 0:512], nps[0][:], rden[:, 0:512])
            nc.sync.dma_start(out=o_ext[:, 0:512], in_=outsb[:, 0:512])
            nc.vector.tensor_mul(
                m3[:, :, 16:32, :], e3[:, :, 16:32, :],
                window_ap(vpE, off, 4, 3, RS, 16, 16))
            mms_for(v, (e4, e3), "den", 1, range(4, K))
            mms_for(v, (m4, m3), "num", 1, range(0, 4))
            mms_for(v, (m4, m3), "num", 1, range(4, K))
            nc.vector.reciprocal_approx_fast(out=rden[:, 512:HW], in_=dps[1][:])
            nc.vector.tensor_mul(outsb[:, 512:HW], nps[1][:], rden[:, 512:HW])
            nc.scalar.dma_start(out=o_ext[:, 512:HW], in_=outsb[:, 512:HW])

    nc.finalize()
    return nc


def _get_nc():
    if "nc" not in _NC_CACHE:
        _NC_CACHE["nc"] = _build_nc()
    return _NC_CACHE["nc"]


def _prep_in_maps(x, wq, wk, wv, rel_h, rel_w):
    bf = ml_dtypes.bfloat16
    ident = np.eye(128, dtype=bf)
    in_maps = []
    for core in range(N_CORES):
        b, cg = divmod(core, 2)
        xb = np.asarray(x[b], dtype=np.float32)
        if cg == 1:
            xb = xb.transpose(0, 2, 1)
        # [ci_within_chunk, ci_chunk, px] so the SBUF partition dim is
        # outermost in DRAM (batched DMAs copy AP-order to AP-order)
        xb = np.ascontiguousarray(
            xb.reshape(2, 128, HW).transpose(1, 0, 2)).astype(np.float16)
        rows = slice(cg * 128, (cg + 1) * 128)
        wt = np.stack([np.asarray(wq)[rows], np.asarray(wk)[rows],
                       np.asarray(wv)[rows]])          # [3, 128, 256]
        wt = wt.transpose(0, 2, 1).astype(np.float16).reshape(
            3, 2, 128, 128)                            # [wi, ci_chunk, ci, co]
        wt = np.ascontiguousarray(
            wt.transpose(2, 0, 1, 3))                  # [ci, wi, ci_chunk, co]
        bias = np.ascontiguousarray(
            np.asarray(rel_h if cg == 0 else rel_w, dtype=np.float32))
        in_maps.append({"x": xb, "w": wt, "bias": bias, "ident": ident})
    return in_maps


def _assemble(results):
    out = np.empty((B, C, H, W), np.float32)
    for core in range(N_CORES):
        b, cg = divmod(core, 2)
        o = results[core]["out"].reshape(128, H, W)
        if cg == 1:
            o = o.transpose(0, 2, 1)
        out[b, cg * 128:(cg + 1) * 128] = o
    return out


def run(inputs, trace=False):
    """Returns (output, BassKernelResults)."""
    from concourse import bass_utils

    nc = _get_nc()
    in_maps = _prep_in_maps(**inputs)
    last_err = None
    for _attempt in range(3):
        try:
            res = bass_utils.run_bass_kernel_spmd(
                nc, in_maps, core_ids=list(range(N_CORES)), trace=trace)
            return _assemble(res.results), res
        except Exception as err:  # transient NRT device errors
            last_err = err
    raise last_err


def kernel(x, wq, wk, wv, rel_h, rel_w):
    out, _ = run(
        dict(x=x, wq=wq, wk=wk, wv=wv, rel_h=rel_h, rel_w=rel_w),
        trace=bool(os.environ.get("ATTNCONV_TRACE")),
    )
    return out
